# revision 57
# baseline (speedup 1.0000x reference)
"""Trainium2 Bass kernel for a dense transformer block (attention + GeGLU-mish
FFN) on x:[2,2048,768], distributed over 8 NeuronCores.

Sharding: core i handles batch i//4, query-block i%4 (512 rows). K/V for the
full 2048-token sequence are computed redundantly per core from a bf16 copy of
x (no collectives, no cross-core sync). All projection matmuls run in fp8e4
with MatmulPerfMode.DoubleRow (two 128-row k-tiles per instruction, 0.5
cycles/row); attention qk runs DoubleRow over [32,2]-split head dims.

Scale folding (fp8 range management, exactly compensated):
  wq,wk x2*g      -> logits x256, compensated inside exp (scale=1/256)
  wv   x8*g       -> wv psum x8; with x4 target activation scale the
                     normalize matmul uses 0.5/den
  wout x8         -> out-proj psum = 32*out, x1 = ps/32 + x
  w1   x8*g       -> psh,psg = 8*(h1,gate); exp(psh/8); mish algebra folds
                     the /64 and the x8 u8 activation scale into one scalar
  w2   x8         -> yps = 64*ffn, y = yps/64 + x1
Attention probs are exp(logit - 3) in fp8 (denominator-normalized, the e^-3
cancels; keeps probs under fp8e4 max). Softmax denominators ride along as a
ones column appended to v. rmsnorm's rsqrt = exp(-0.5*ln(ms+eps)) so every
activation lives in the one natural_log_exp table (no act-table switches).
"""
import sys

sys.path.insert(0, "/opt/trn_rl_repo")

import numpy as np
import ml_dtypes

import bass_rust
import concourse.bass as bass
import concourse.mybir as mybir
import concourse.tile as tile
from concourse.bass_utils import run_bass_kernel_spmd

AF = mybir.ActivationFunctionType
ALU = mybir.AluOpType
BF16 = mybir.dt.bfloat16
F32 = mybir.dt.float32
FP8 = mybir.dt.float8e4
DR = mybir.MatmulPerfMode.DoubleRow

DIM = 768
NH = 12
HD = 64
HIDDEN = 3072
S = 2048
QB = 512
EPS = 1e-5
# exp-as-fp8-bits: uint8(x*A+B) bitcast to fp8e4 ~ e^(x/256-3); used on a
# subset of attention prob tiles to split exp work between Act and DVE
A_EXP = 8.0 / (256.0 * float(np.log(2.0)))
B_EXP = 56.0 - 24.0 / float(np.log(2.0))
NCK = DIM // 128   # 6 chunks of the model dim
NDP = NCK // 2     # 3 DoubleRow chunk-pairs
NT = S // 128      # 16 token slices
KTP = 8            # key-block pairs of 256

# ---------------------------------------------------------------------------
# Workaround for a walrus codegen limit: an instruction may carry at most one
# sync-wait command, but TileContext's exit drain accumulates one wait per
# logical proc. Split the waits onto chained same-engine NOPs.
# ---------------------------------------------------------------------------


def _split_waits(nc):
    for f in nc.m.functions:
        for bb in f.blocks:
            snapshot = list(bb.instructions)
            new = []
            for inst in snapshot:
                si = inst.sync_info
                waits = list(si.on_wait) if si and si.on_wait else []
                limit = 1
                if len(waits) > limit:
                    si.on_wait = waits[:limit]
                    eng = nc.engines[inst.engine]
                    for w in waits[limit:]:
                        nop = eng.nop()
                        popped = nc.cur_bb.bb.instructions.pop()
                        assert popped is nop.ins
                        nop.ins.sync_info = bass_rust.SyncInfo(
                            on_wait=[w], on_update=[])
                        new.append(nop.ins)
                new.append(inst)
            bb.instructions[:] = new


# ---------------------------------------------------------------------------
# Device program
# ---------------------------------------------------------------------------

def build_nc(repeat=1, coll=True, debug=False):
    nc = bass.Bass()
    xo_d = nc.dram_tensor("xo", [DIM, QB], F32, kind="ExternalInput")
    xf_d = nc.dram_tensor("xf", [DIM, S], BF16, kind="ExternalInput")
    wqk_d = nc.dram_tensor("wqk", [12, 128, NDP, 2, 128], FP8,
                           kind="ExternalInput")
    wvm_d = nc.dram_tensor("wvm", [NDP, 128, 2, DIM], FP8,
                           kind="ExternalInput")
    wo_d = nc.dram_tensor("wo", [NCK, 128, NDP, 2, 128], FP8,
                          kind="ExternalInput")
    w1_d = nc.dram_tensor("w1", [48, 128, NDP, 2, 128], FP8,
                          kind="ExternalInput")
    w2_d = nc.dram_tensor("w2", [NCK, 128, 12, 2, 128], FP8,
                          kind="ExternalInput")
    yT_d = nc.dram_tensor("yT", [DIM, QB], F32, kind="ExternalOutput")
    dbg = None
    if debug:
        dbg = {
            "xh8o": nc.dram_tensor("dbg_xh8o", [128, 2, QB], FP8,
                                   kind="ExternalOutput"),
            "qT8": nc.dram_tensor("dbg_qT8", [128, QB], FP8,
                                  kind="ExternalOutput"),
            "kT8": nc.dram_tensor("dbg_kT8", [128, S], FP8,
                                  kind="ExternalOutput"),
            "k8p": nc.dram_tensor("dbg_k8p", [128, 4, S], FP8,
                                  kind="ExternalOutput"),
            "v8": nc.dram_tensor("dbg_v8", [128, 2, NH, 128], FP8,
                                 kind="ExternalOutput"),
            "wv8": nc.dram_tensor("dbg_wv8", [128, 2, QB], FP8,
                                  kind="ExternalOutput"),
            "x1": nc.dram_tensor("dbg_x1", [128, QB], F32,
                                 kind="ExternalOutput"),
            "u8": nc.dram_tensor("dbg_u8", [128, 2, QB], FP8,
                                 kind="ExternalOutput"),
        }

    with tile.TileContext(nc) as tc, \
         nc.allow_low_precision(reason="fp8/bf16 kernel"):
        for _ in range(repeat):
            _body(nc, tc, xo_d, xf_d, wqk_d, wvm_d, wo_d, w1_d, w2_d, yT_d,
                  dbg=dbg)
    _split_waits(nc)
    return nc


def _norm(nc, pools, sq_src, n_tok, ones_col, eps1, out_write):
    """rmsnorm: squares -> column-sum matmul -> exp(-0.5*ln(ms+eps)) ->
    broadcast matmul -> out_write(c, rbc_psum). sq_src(c) gives the [128,n_tok]
    input AP for chunk c. Fixed tile names so pools ring-reuse buffers.
    Squares run on Pool (sbuf-only inputs) to keep DVE free."""
    norm_sb, ss_ps, rbc_ps, row_ones = pools
    ss = ss_ps.tile([1, n_tok], F32, name="ss")
    for c in range(NCK):
        sq = norm_sb.tile([128, n_tok], BF16, name="sq")
        eng = nc.vector if c % 3 == 0 else nc.gpsimd
        eng.tensor_tensor(sq[:], sq_src(c), sq_src(c), ALU.mult)
        nc.tensor.matmul(ss[:], ones_col[:], sq[:],
                         start=(c == 0), stop=(c == NCK - 1))
    rln = norm_sb.tile([1, n_tok], F32, name="rln")
    nc.scalar.activation(out=rln[:], in_=ss[:], func=AF.Ln,
                         bias=eps1[:], scale=1.0 / DIM)
    rstd = norm_sb.tile([1, n_tok], BF16, name="rstd")
    nc.scalar.activation(out=rstd[:], in_=rln[:], func=AF.Exp, scale=-0.5)
    rbc = rbc_ps.tile([128, n_tok], F32, name="rbc")
    nc.tensor.matmul(rbc[:], row_ones[:], rstd[:], start=True, stop=True)
    for c in range(NCK):
        out_write(c, rbc)


def _body(nc, tc, xo_d, xf_d, wqk_d, wvm_d, wo_d, w1_d, w2_d, yT_d, dbg=None):
    from contextlib import ExitStack

    ctx = ExitStack()
    with ctx:
        singles = ctx.enter_context(tc.tile_pool(name="singles", bufs=1))

        ones_col = singles.tile([128, 1], BF16)
        nc.vector.memset(ones_col[:], 1.0)
        row_ones = singles.tile([1, 128], BF16)
        nc.vector.memset(row_ones[:], 1.0)
        # broadcast row for the attention normalizer, placed at partition 64
        # (the den row of the wv psum lives there; lanes are locked)
        mham = singles.tile([65, 64], BF16)  # 0.5 = v-scale(1/8) * wv8 act x4
        nc.vector.memset(mham[64:65, :], 0.5)
        eps1 = singles.tile([1, 1], F32)
        nc.vector.memset(eps1[:], EPS)
        bm3 = singles.tile([128, 1], F32)
        nc.vector.memset(bm3[:], -3.0)
        b2 = singles.tile([128, 1], F32)
        nc.vector.memset(b2[:], 2.0)
        b18 = singles.tile([128, 1], F32)
        nc.vector.memset(b18[:], 0.125)
        c2 = singles.tile([128, QB], BF16)  # Pool has no tensor_scalar
        nc.vector.memset(c2[:], 2.0)

        # persistent activations / resident weights
        x_own = [singles.tile([128, QB], F32, name=f"xo_{c}")
                 for c in range(NCK)]
        for c in range(NCK):
            eng = nc.gpsimd if c % 2 else nc.sync
            eng.dma_start(x_own[c][:], xo_d[c * 128:(c + 1) * 128, :])
        wqk_sb = [singles.tile([128, NDP, 2, 128], FP8, name=f"wqk_{i}")
                  for i in range(12)]
        for i in range(12):
            nc.sync.dma_start(wqk_sb[i][:], wqk_d[i])
        wvm_sb = [singles.tile([128, 2, DIM], FP8, name=f"wvm_{i}")
                  for i in range(NDP)]
        for i in range(NDP):
            nc.sync.dma_start(wvm_sb[i][:], wvm_d[i])
        x1 = [singles.tile([128, QB], F32, name=f"x1_{c}")
              for c in range(NCK)]
        xh18 = [singles.tile([128, 2, QB], FP8, name=f"xh18_{i}")
                for i in range(NDP)]
        wv8 = [singles.tile([128, 2, QB], FP8, name=f"wv8_{i}")
               for i in range(NDP)]
        u8 = [singles.tile([128, 2, QB], FP8, name=f"u8_{j}")
              for j in range(12)]
        wo_sb = [singles.tile([128, NDP, 2, 128], FP8, name=f"wo_{i}")
                 for i in range(NCK)]
        w2_sb = [singles.tile([128, 12, 2, 128], FP8, name=f"w2_{i}")
                 for i in range(NCK)]

        # attention operands (die after attention)
        att = ctx.enter_context(tc.tile_pool(name="att", bufs=1))
        # per-head stationary padded to 128 (DoubleRow needs M in {64,128}):
        # dims 0-63 = v, 64 = ones (softmax denominator), 65-127 = garbage
        # that lands in never-read psum rows
        v8 = [att.tile([128, 2, NH, 128], FP8, name=f"v8_{t}")
              for t in range(KTP)]
        for t in range(KTP):
            nc.gpsimd.memset(v8[t][:, :, :, HD:HD + 1], 1.0)
        k8p = [att.tile([128, 4, S], FP8, name=f"k8_{i}") for i in range(2)]
        q8p = [att.tile([128, 4, QB], FP8, name=f"q8_{i}") for i in range(2)]

        # ---------------- phase 1+2: norms, q/k/v projections -------------
        with tc.tile_pool(name="ph12", bufs=1) as ph12, \
             tc.tile_pool(name="nsb", bufs=2) as norm_sb, \
             tc.tile_pool(name="ssps", bufs=2, space="PSUM") as ss_ps, \
             tc.tile_pool(name="rbps", bufs=2, space="PSUM") as rbc_ps, \
             tc.tile_pool(name="qkps", bufs=2, space="PSUM") as qk_ps, \
             tc.tile_pool(name="vps", bufs=1, space="PSUM") as v_ps:
            npools = (norm_sb, ss_ps, rbc_ps, row_ones)
            xh8o = [ph12.tile([128, 2, QB], FP8, name=f"xh8o_{i}")
                    for i in range(NDP)]
            xh8f = [ph12.tile([128, 2, S], FP8, name=f"xh8f_{i}")
                    for i in range(NDP)]
            xf = [ph12.tile([128, S], BF16, name=f"xf_{c}")
                  for c in range(NCK)]
            for c in range(NCK):
                nc.sync.dma_start(xf[c][:], xf_d[c * 128:(c + 1) * 128, :])
            kT8 = [ph12.tile([128, S], FP8, name=f"kT8_{c}")
                   for c in range(NCK)]
            qT8 = [ph12.tile([128, QB], FP8, name=f"qT8_{c}")
                   for c in range(NCK)]

            # norm1 on own block -> xh8o
            def wr_own(c, rbc):
                nc.vector.tensor_tensor(
                    xh8o[c // 2][:, c % 2, :], x_own[c][:], rbc[:], ALU.mult)
            _norm(nc, npools, lambda c: x_own[c][:], QB, ones_col, eps1,
                  wr_own)

            # q projection (own block)
            for oc in range(NCK):
                ps = qk_ps.tile([128, QB], F32, name="qkp")
                for dp in range(NDP):
                    nc.tensor.matmul(ps[:], wqk_sb[oc][:, dp],
                                     xh8o[dp][:], start=(dp == 0),
                                     stop=(dp == NDP - 1), perf_mode=DR)
                nc.vector.tensor_copy(qT8[oc][:], ps[:])

            # norm1 full sequence (per token-block) -> xh8f, then k and v
            for tb in range(4):
                sl = slice(tb * QB, (tb + 1) * QB)

                def wr_full(c, rbc, sl=sl):
                    nc.vector.tensor_tensor(
                        xh8f[c // 2][:, c % 2, sl], xf[c][:, sl], rbc[:],
                        ALU.mult)
                _norm(nc, npools, lambda c, sl=sl: xf[c][:, sl], QB,
                      ones_col, eps1, wr_full)

                for oc in range(NCK):
                    ps = qk_ps.tile([128, QB], F32, name="qkp")
                    for dp in range(NDP):
                        nc.tensor.matmul(ps[:], wqk_sb[6 + oc][:, dp],
                                         xh8f[dp][:, :, sl], start=(dp == 0),
                                         stop=(dp == NDP - 1), perf_mode=DR)
                    nc.scalar.copy(out=kT8[oc][:, sl], in_=ps[:])
                for t in range(tb * 4, tb * 4 + 4):
                    ps = v_ps.tile([128, DIM], F32, name="vps")
                    tsl = slice(t * 128, (t + 1) * 128)
                    # split on the free dim: a matmul output cannot cross a
                    # 2KB PSUM bank boundary
                    for off, width in ((0, 512), (512, 256)):
                        for dp in range(NDP):
                            nc.tensor.matmul(
                                ps[:, off:off + width],
                                xh8f[dp][:, :, tsl],
                                wvm_sb[dp][:, :, off:off + width],
                                start=(dp == 0), stop=(dp == NDP - 1),
                                perf_mode=DR)
                    nc.vector.tensor_copy(
                        v8[t // 2][:, t % 2, :, 0:HD],
                        ps.rearrange("p (h d) -> p h d", h=NH))

            # repack k/q into [32,2]-contraction DoubleRow layout.
            # One DMA per 32-partition group: partition-permuting APs inside
            # a single DMA produce garbage, partition slices are fine.
            for c in range(NCK):
                ti, tb_base = divmod(c, 4)
                base = 32 * tb_base
                for hs in range(4):
                    nc.sync.dma_start(
                        k8p[ti][base:base + 32, hs, :],
                        kT8[c][hs * 32:(hs + 1) * 32, :])
                    nc.sync.dma_start(
                        q8p[ti][base:base + 32, hs, :],
                        qT8[c][hs * 32:(hs + 1) * 32, :])
            if dbg is not None:
                nc.gpsimd.dma_start(dbg["xh8o"][...], xh8o[0][:])
                nc.gpsimd.dma_start(dbg["qT8"][...], qT8[0][:])
                nc.gpsimd.dma_start(dbg["kT8"][...], kT8[0][:])

        # deferred weight loads (overlap with attention)
        for i in range(NCK):
            nc.sync.dma_start(wo_sb[i][:], wo_d[i])
        for i in range(NCK):
            nc.sync.dma_start(w2_sb[i][:], w2_d[i])
        if dbg is not None:
            nc.gpsimd.dma_start(dbg["k8p"][...], k8p[0][:])
            nc.gpsimd.dma_start(dbg["v8"][...], v8[0][:])

        # ---------------- phase 3: attention ------------------------------
        with tc.tile_pool(name="apT", bufs=8) as pT_p, \
             tc.tile_pool(name="asb", bufs=3) as att_sb, \
             tc.tile_pool(name="aps", bufs=2, space="PSUM") as ps_p, \
             tc.tile_pool(name="awv", bufs=2, space="PSUM") as wv_p, \
             tc.tile_pool(name="arec", bufs=2, space="PSUM") as rec_p:
            for c in range(NCK):
                ti, tb_base = divmod(c, 4)
                base = 32 * tb_base
                wvh = [wv_p.tile([128, QB], F32, name="wvps")
                       for _ in range(2)]
                for ktp in range(KTP):
                    for h in range(2):
                        ps = ps_p.tile([128, 2 * QB], F32, name="sAB")
                        for sub in range(2):
                            ksl = slice(ktp * 256 + sub * 128,
                                        ktp * 256 + (sub + 1) * 128)
                            nc.tensor.matmul(
                                ps[:, sub * QB:(sub + 1) * QB],
                                k8p[ti][base:base + 32, 2 * h:2 * h + 2, ksl],
                                q8p[ti][base:base + 32, 2 * h:2 * h + 2, :],
                                start=True, stop=True, perf_mode=DR,
                                tile_position=(base, 0))
                        i_t = 2 * ktp + h
                        if i_t % 2 == 1 and i_t < 12:
                            pb = pT_p.tile([128, 2 * QB], mybir.dt.uint8,
                                           name="pTb")
                            nc.vector.tensor_scalar(pb[:], ps[:], A_EXP,
                                                    B_EXP, ALU.mult, ALU.add)
                            pT_ap = pb[:].bitcast(FP8)
                        else:
                            pT = pT_p.tile([128, 2 * QB], FP8, name="pT")
                            nc.scalar.activation(out=pT[:], in_=ps[:],
                                                 func=AF.Exp, bias=bm3[:],
                                                 scale=1.0 / 256.0)
                            pT_ap = pT[:]
                        nc.tensor.matmul(
                            wvh[h][:, :],
                            v8[ktp][:, :, 2 * c + h, :],
                            pT_ap.rearrange("p (s q) -> p s q", s=2),
                            start=(ktp == 0), stop=(ktp == KTP - 1),
                            perf_mode=DR)
                # normalize: per-head reciprocal denominators broadcast.
                # The den row lives at partition 64 (lane-locked), so the
                # recip stays there and the broadcast matmul reads row 64.
                for h in range(2):
                    den = att_sb.tile([65, QB], BF16, name="den")
                    nc.vector.reciprocal(den[64:65, :], wvh[h][HD:HD + 1, :])
                    rec = rec_p.tile([64, QB], F32, name="rec")
                    nc.tensor.matmul(rec[:], mham[64:65, :], den[64:65, :],
                                     start=True, stop=True,
                                     tile_position=(64, 0))
                    rsb = att_sb.tile([64, QB], BF16, name="rsb")
                    nc.vector.tensor_copy(rsb[:], rec[:])
                    if h == 0:
                        nc.vector.tensor_tensor(
                            wv8[c // 2][0:HD, c % 2, :], wvh[h][0:HD, :],
                            rsb[:], ALU.mult)
                    else:
                        tmp = att_sb.tile([64, QB], FP8, name="wvtmp")
                        nc.vector.tensor_tensor(
                            tmp[:], wvh[h][0:HD, :], rsb[:], ALU.mult)
                        nc.gpsimd.dma_start(
                            wv8[c // 2][HD:128, c % 2, :], tmp[:])

        if dbg is not None:
            nc.gpsimd.dma_start(dbg["wv8"][...], wv8[0][:])

        # ---------------- phase 4: out-proj + residual + norm2 ------------
        with tc.tile_pool(name="nsb2", bufs=2) as norm_sb2, \
             tc.tile_pool(name="ssp2", bufs=2, space="PSUM") as ss_ps2, \
             tc.tile_pool(name="rbp2", bufs=2, space="PSUM") as rbc_ps2, \
             tc.tile_pool(name="op_ps", bufs=3, space="PSUM") as op_ps:
            for oc in range(NCK):
                ps = op_ps.tile([128, QB], F32, name="op")
                for dp in range(NDP):
                    nc.tensor.matmul(ps[:], wo_sb[oc][:, dp], wv8[dp][:],
                                     start=(dp == 0), stop=(dp == NDP - 1),
                                     perf_mode=DR)
                nc.vector.scalar_tensor_tensor(
                    x1[oc][:], ps[:], 1.0 / 32.0, x_own[oc][:],
                    ALU.mult, ALU.add)

            def wr_n2(c, rbc):
                nc.vector.tensor_tensor(
                    xh18[c // 2][:, c % 2, :], x1[c][:], rbc[:], ALU.mult)
            _norm(nc, (norm_sb2, ss_ps2, rbc_ps2, row_ones),
                  lambda c: x1[c][:], QB, ones_col, eps1, wr_n2)

        if dbg is not None:
            nc.gpsimd.dma_start(dbg["x1"][...], x1[0][:])

        # ---------------- phase 5: FFN ------------------------------------
        w1s = ctx.enter_context(tc.tile_pool(name="w1s", bufs=6))
        ffn = ctx.enter_context(tc.tile_pool(name="ffn", bufs=3))
        pre_w1 = {}
        for j in range(2):
            wh = w1s.tile([128, NDP, 2, 128], FP8, name="w1h")
            nc.sync.dma_start(wh[:], w1_d[j])
            wg = w1s.tile([128, NDP, 2, 128], FP8, name="w1g")
            nc.sync.dma_start(wg[:], w1_d[24 + j])
            pre_w1[j] = (wh, wg)

        for half in range(2):
            with tc.tile_pool(name=f"y_ps{half}", bufs=1,
                              space="PSUM") as y_ps, \
                 tc.tile_pool(name=f"f_psh{half}", bufs=3,
                              space="PSUM") as f_psh, \
                 tc.tile_pool(name=f"f_psg{half}", bufs=2,
                              space="PSUM") as f_psg:
                yps = y_ps.tile([128, 3 * QB], F32, name="y")
                for j in range(24):
                    if half == 0:
                        if j in pre_w1:
                            wh, wg = pre_w1[j]
                        else:
                            wh = w1s.tile([128, NDP, 2, 128], FP8, name="w1h")
                            nc.sync.dma_start(wh[:], w1_d[j])
                            wg = w1s.tile([128, NDP, 2, 128], FP8, name="w1g")
                            nc.sync.dma_start(wg[:], w1_d[24 + j])
                        psh = f_psh.tile([128, QB], F32, name="psh")
                        psg = f_psg.tile([128, QB], F32, name="psg")
                        for dp in range(NDP):
                            nc.tensor.matmul(psh[:], wh[:, dp], xh18[dp][:],
                                             start=(dp == 0),
                                             stop=(dp == NDP - 1),
                                             perf_mode=DR)
                        for dp in range(NDP):
                            nc.tensor.matmul(psg[:], wg[:, dp], xh18[dp][:],
                                             start=(dp == 0),
                                             stop=(dp == NDP - 1),
                                             perf_mode=DR)
                        # u8 = psh*psg*T/8, T = 1 - 2/((1+e^{psh/8})^2+1)
                        me = ffn.tile([128, QB], BF16, name="me")
                        nc.scalar.activation(out=me[:], in_=psh[:],
                                             func=AF.Exp, scale=1.0 / 8.0)
                        a2 = ffn.tile([128, QB], BF16, name="mish_a")
                        nc.gpsimd.tensor_tensor(a2[:], me[:], c2[:], ALU.add)
                        s = ffn.tile([128, QB], BF16, name="mish_s")
                        nc.gpsimd.tensor_tensor(s[:], a2[:], me[:], ALU.mult)
                        d = ffn.tile([128, QB], BF16, name="mish_d")
                        nc.scalar.activation(out=d[:], in_=s[:],
                                             func=AF.Identity, bias=b2[:])
                        r = ffn.tile([128, QB], BF16, name="mish_r")
                        nc.vector.reciprocal(r[:], d[:])
                        w = ffn.tile([128, QB], BF16, name="mish_w")
                        nc.scalar.activation(out=w[:], in_=r[:],
                                             func=AF.Identity, bias=b18[:],
                                             scale=-0.25)
                        t1 = ffn.tile([128, QB], BF16, name="mish_t1")
                        nc.vector.tensor_tensor(t1[:], psg[:], w[:], ALU.mult)
                        nc.vector.tensor_tensor(
                            u8[j // 2][:, j % 2, :], psh[:], t1[:], ALU.mult)
                    if j % 2 == 1:
                        jp = j // 2
                        for o3 in range(3):
                            oc = half * 3 + o3
                            nc.tensor.matmul(
                                yps[:, o3 * QB:(o3 + 1) * QB],
                                w2_sb[oc][:, jp], u8[jp][:],
                                start=(jp == 0), stop=(jp == 11),
                                perf_mode=DR, skip_group_check=True)
                if dbg is not None and half == 0:
                    nc.gpsimd.dma_start(dbg["u8"][...], u8[0][:])
                for o3 in range(3):
                    oc = half * 3 + o3
                    yout = ffn.tile([128, QB], F32, name="yout")
                    nc.vector.scalar_tensor_tensor(
                        yout[:], yps[:, o3 * QB:(o3 + 1) * QB], 1.0 / 64.0,
                        x1[oc][:], ALU.mult, ALU.add)
                    nc.sync.dma_start(
                        yT_d[oc * 128:(oc + 1) * 128, :], yout[:])


# ---------------------------------------------------------------------------
# Host wrapper
# ---------------------------------------------------------------------------

_NC_CACHE = {}


def _get_nc():
    if "nc" not in _NC_CACHE:
        _NC_CACHE["nc"] = build_nc()
    return _NC_CACHE["nc"]


def _dr_stationary(W, n_oc, fp8):
    """W [768, n_oc*128] -> [n_oc, 128, 3, 2, 128] DoubleRow stationary."""
    Wr = np.asarray(W, np.float32).reshape(NDP, 2, 128, n_oc, 128)
    return np.ascontiguousarray(Wr.transpose(3, 2, 0, 1, 4)).astype(fp8)


def _prep_inputs(x, w_qkv, w_out, w1, w2, g_attn, g_ff):
    fp8 = ml_dtypes.float8_e4m3
    bf16 = ml_dtypes.bfloat16
    wq = (g_attn[:, None] * w_qkv[:, :DIM]) * 2.0
    wk = (g_attn[:, None] * w_qkv[:, DIM:2 * DIM]) * 2.0
    wv = (g_attn[:, None] * w_qkv[:, 2 * DIM:]) * 8.0
    wqk8 = np.concatenate(
        [_dr_stationary(wq, NCK, fp8), _dr_stationary(wk, NCK, fp8)], axis=0)
    wvm8 = np.ascontiguousarray(
        np.asarray(wv, np.float32).reshape(NDP, 2, 128, DIM)
        .transpose(0, 2, 1, 3)).astype(fp8)
    wo8 = _dr_stationary(np.asarray(w_out, np.float32) * 8.0, NCK, fp8)
    w1f = (g_ff[:, None] * w1) * 8.0
    w18 = np.concatenate(
        [_dr_stationary(w1f[:, :HIDDEN], 24, fp8),
         _dr_stationary(w1f[:, HIDDEN:], 24, fp8)], axis=0)
    w2r = (np.asarray(w2, np.float32) * 8.0).reshape(12, 2, 128, NCK, 128)
    w28 = np.ascontiguousarray(w2r.transpose(3, 2, 0, 1, 4)).astype(fp8)

    in_maps = []
    for core in range(8):
        b, qb = core // 4, core % 4
        xo = np.ascontiguousarray(
            x[b][qb * QB:(qb + 1) * QB].T.astype(np.float32))
        xfb = np.ascontiguousarray(x[b].T.astype(bf16))
        in_maps.append({
            "xo": xo, "xf": xfb, "wqk": wqk8, "wvm": wvm8,
            "wo": wo8, "w1": w18, "w2": w28,
        })
    return in_maps


def run(x, w_qkv, w_out, w1, w2, g_attn, g_ff, trace=False, **kw):
    nc = _get_nc()
    in_maps = _prep_inputs(x, w_qkv, w_out, w1, w2, g_attn, g_ff)
    res = run_bass_kernel_spmd(
        nc, in_maps, core_ids=list(range(8)), trace=trace, **kw)
    B = x.shape[0]
    y = np.zeros((B, S, DIM), dtype=np.float32)
    for core in range(8):
        b, qb = core // 4, core % 4
        yT = res.results[core]["yT"]  # [768, 512]
        y[b, qb * QB:(qb + 1) * QB, :] = np.asarray(yT).T
    return y, res


def kernel(x, w_qkv, w_out, w1, w2, g_attn, g_ff):
    y, _ = run(np.asarray(x, np.float32), np.asarray(w_qkv, np.float32),
               np.asarray(w_out, np.float32), np.asarray(w1, np.float32),
               np.asarray(w2, np.float32), np.asarray(g_attn, np.float32),
               np.asarray(g_ff, np.float32))
    return y
